# revision 27
# baseline (speedup 1.0000x reference)
"""Trainium2 Bass kernel for nn_MultiHeadedAttention_41583873359904.

Reference computation (B=8, C=256, H=W=128):
  q/k/v = 1x1 conv projections of x/y/z
  scores[b,c,h,h'] = q[b,c,h,:].k[b,c,h',:]/sqrt(W); p = softmax(scores, -1)
  att = p @ v  (per b,c)
  o = conv3x3(att) + b_out -> BatchNorm2d(batch stats) -> LeakyReLU(0.2)

Sharding: data-parallel over batch, one batch element per NeuronCore (8 cores).
BatchNorm batch stats are combined with on-device AllReduces of per-core
(sum, sumsq), split per 128-channel chunk so the first chunk's normalize+store
overlaps the second chunk's convolution.

Per-core layout strategy:
  - V projection channel-major ([oc, pix]) -> DRAM planes, then one big
    SBUF-resident copy v_sb[w? no: h'(part), c, W+1] with a fused ones
    column so each attention matmul also produces the softmax denominator.
  - Q/K projections pixel-major ([pix, oc]) via fp8 DoubleRow matmuls
    (x/y and wq/wk quantized e4m3 host-side); Q_sb/K_sb stored fp8
    [w, c, h]; scores^T = K^T.T @ Q^T per channel; exp on ACT with the
    dequant scale folded in; att[h, w|den] = E^T.T @ [v|1].
  - bk is dropped entirely (per-channel constant shift of k is
    softmax-invariant); bq is applied by a fused DVE op on the PSUM copy.
  - att planes written zero-padded [C,130,130] to DRAM; 3x3 conv = 18
    accumulated matmuls per [128oc, 512pix] PSUM tile, occ-outer so BN
    stats AllReduce + normalize of chunk 0 hide under chunk 1's conv.
  - b_out is dropped (BatchNorm cancels any per-channel bias); attention
    scale 1/sqrt(W) is folded into wq/bq on the host.
  - LeakyReLU(0.2) of y = s*x+t as max(0.2*y, y) in one fused DVE op.

Matmul operands bf16 except the q/k path (fp8 e4m3, TRN max 240);
fp32 PSUM accumulation everywhere.
"""

import math

import numpy as np
import ml_dtypes

import concourse.bass as bass
import concourse.tile as tile
from concourse import mybir
from concourse import tile_sem_assignment as _tsa
from concourse.tile import ScopedClock as _ScopedClock
from concourse.bass_utils import run_bass_kernel_spmd

B, C, H, W = 8, 256, 128, 128
HW = H * W          # 16384 pixels per plane
PB = 512            # pixels per conv/proj tile (4 rows)
NB = HW // PB       # 32 pixel blocks
CH = C // 128       # 2 channel chunks of 128
BN_EPS = 1e-5
LEAKY = 0.2
N_CORES = 8
N_TOT = float(B * HW)   # BN element count per channel

BF16 = mybir.dt.bfloat16
F32 = mybir.dt.float32
FP8 = mybir.dt.float8e4
nbf16 = ml_dtypes.bfloat16
nfp8 = ml_dtypes.float8_e4m3

# fp8 scale plumbing: x,y scaled by SX host-side; wq,wk scaled by SW.
# PSUM q = q_true * SX*SW; stored Q_sb = q_true * SQ (copy scale SQ/(SX*SW)).
SX = 32.0
SW = 1024.0
SQ = 128.0
COPY_SCALE = SQ / (SX * SW)      # PSUM -> Q_sb/K_sb dequant-requant
EXP_SCALE = 1.0 / (SQ * SQ)      # scores PSUM -> true scores for exp


class _SplitDrainTileContext(tile.TileContext):
    """The walrus in this container rejects >1 sync wait per instruction.
    Tile routinely emits several (RAW + WAR). Hoist extra waits onto NOPs
    committed immediately before on the same engine (sequencers execute in
    order, so waiting on the NOPs first is equivalent), and split the tail
    drain's global-clock waits the same way."""

    def _commit_instruction(self, inst, lazy_reg_writes=True):
        si = getattr(inst, "sync_info", None)
        if (
            si is not None
            and si.on_wait
            and len(si.on_wait) > 1
            and inst.engine != mybir.EngineType.Unassigned
            and not isinstance(inst, mybir.InstUnconditionalBranch)
        ):
            waits = list(si.on_wait)
            for w in waits[:-1]:
                nop = mybir.InstNoOp(
                    name=self.nc.get_next_instruction_name(),
                    engine=inst.engine,
                    ins=[],
                    outs=[],
                    sync_info=mybir.SyncInfo(on_wait=[w], on_update=[]),
                    bass_nofuse=True,
                )
                super()._commit_instruction(nop, lazy_reg_writes=False)
            inst.sync_info = mybir.SyncInfo(
                on_wait=[waits[-1]], on_update=list(si.on_update or [])
            )
        super()._commit_instruction(inst, lazy_reg_writes)

    def _drain_and_barrier(self, tick_clock, wait_clock):
        nc = self.nc
        gc = tick_clock.global_clock
        procs = [(p, gc.peek_next(p) - 1) for p in range(_tsa.N_PROCS)]
        for p, t in procs:
            if t <= 0:
                continue
            sub = _tsa.VectorClock()
            sub.require_at_least(p, t)
            nop = nc.sync.nop(nofuse=True, hint="split_drain_wait")
            wait_clock.add_sem_waits(nop.ins, _ScopedClock({None: sub}))
        nc.sync.drain()
        nc.all_engine_barrier()
        assert self.sems is not None
        popped = nc._tile_sem_poison_stack.pop()
        assert popped is self._sem_poison
        nc.clear_and_free_semaphores(list(self.sems.allocated().values()))
        nc.all_engine_barrier()


def _build():
    nc = bass.Bass(num_devices=N_CORES)

    # Per-core external inputs (host wrapper prepares dtype/layout/scales).
    xb = nc.dram_tensor("xb", [C, HW], FP8, kind="ExternalInput")
    yb = nc.dram_tensor("yb", [C, HW], FP8, kind="ExternalInput")
    zb = nc.dram_tensor("zb", [C, HW], BF16, kind="ExternalInput")
    wq8 = nc.dram_tensor("wq8", [C, C], FP8, kind="ExternalInput")   # [ic,oc]
    wk8 = nc.dram_tensor("wk8", [C, C], FP8, kind="ExternalInput")
    wvT = nc.dram_tensor("wvT", [C, C], BF16, kind="ExternalInput")
    bqb = nc.dram_tensor("bqb", [128, C], F32, kind="ExternalInput")  # bq*SQ bcast
    bv = nc.dram_tensor("bv", [C, 1], F32, kind="ExternalInput")
    wtap = nc.dram_tensor("wtap", [9 * CH, 128, C], BF16, kind="ExternalInput")
    gamma = nc.dram_tensor("gamma", [C, 1], F32, kind="ExternalInput")
    beta = nc.dram_tensor("beta", [C, 1], F32, kind="ExternalInput")

    out = nc.dram_tensor("out", [C, HW], BF16, kind="ExternalOutput")

    # DRAM scratch
    v_dram = nc.dram_tensor("v_scratch", [C, HW], BF16)
    att_dram = nc.dram_tensor("att_scratch", [C, H + 2, W + 2], BF16)

    with _SplitDrainTileContext(nc) as tc:
        with tc.tile_pool(name="singles", bufs=1) as singles:
            # ---- constants / weights ----
            eps_sb = singles.tile([128, 1], F32)
            nc.vector.memset(eps_sb, BN_EPS)
            zrow = singles.tile([128, W + 2], BF16)
            nc.vector.memset(zrow, 0.0)
            for cc in range(CH):
                nc.sync.dma_start(out=att_dram[cc * 128:(cc + 1) * 128, 0, :], in_=zrow)
                nc.sync.dma_start(out=att_dram[cc * 128:(cc + 1) * 128, H + 1, :], in_=zrow)

            wv_sb = singles.tile([128, CH, C], BF16)   # [ic(part), icc, oc]
            nc.scalar.dma_start(out=wv_sb, in_=wvT.rearrange("(a p) c -> p a c", p=128))
            bv_sb = singles.tile([128, CH], F32)
            nc.scalar.dma_start(out=bv_sb, in_=bv.rearrange("(a p) o -> p (a o)", p=128))
            wq_sb = singles.tile([128, CH, C], FP8)
            nc.sync.dma_start(out=wq_sb, in_=wq8.rearrange("(a p) c -> p a c", p=128))
            wk_sb = singles.tile([128, CH, C], FP8)
            nc.sync.dma_start(out=wk_sb, in_=wk8.rearrange("(a p) c -> p a c", p=128))
            bq_sb = singles.tile([128, C], F32)
            nc.sync.dma_start(out=bq_sb, in_=bqb[:, :])
            wt_sb = singles.tile([128, 9 * CH, C], BF16)
            nc.sync.dma_start(out=wt_sb, in_=wtap.rearrange("t p c -> p t c"))
            g_sb = singles.tile([128, CH], F32)
            nc.gpsimd.dma_start(out=g_sb, in_=gamma.rearrange("(a p) o -> p (a o)", p=128))
            be_sb = singles.tile([128, CH], F32)
            nc.gpsimd.dma_start(out=be_sb, in_=beta.rearrange("(a p) o -> p (a o)", p=128))

            zb_r = zb.rearrange("(a p) x -> p a x", p=128)
            xb_r = xb.rearrange("(a p) x -> p a x", p=128)
            yb_r = yb.rearrange("(a p) x -> p a x", p=128)
            v_out_r = v_dram.rearrange("(a p) x -> p a x", p=128)

            # =============== Phase 1: interleaved V-proj + Q/K-proj ==========
            with tc.tile_pool(name="qk_store", bufs=1) as qkstore:
                Q_sb = qkstore.tile([128, H, C], FP8)   # [w, h, c] (c contiguous)
                K_sb = qkstore.tile([128, H, C], FP8)
                v_sb = qkstore.tile([128, C, W + 2], BF16)  # [h', c, w|1|pad]

                with tc.tile_pool(name="p1_in", bufs=4) as p1in, \
                     tc.tile_pool(name="p1_vo", bufs=4) as p1vo, \
                     tc.tile_pool(name="p1_vps", bufs=3, space="PSUM") as vps, \
                     tc.tile_pool(name="p1_qkps", bufs=3, space="PSUM") as qkps:
                    nc.gpsimd.memset(v_sb[:, :, W:W + 1], 1.0)
                    v_in_r = v_dram.rearrange("c (h w) -> h c w", w=W)

                    def v_block(pb):
                        zt = p1in.tile([128, CH, PB], BF16, tag="zt")
                        nc.gpsimd.dma_start(out=zt, in_=zb_r[:, :, pb * PB:(pb + 1) * PB])
                        for occ in range(CH):
                            ps = vps.tile([128, PB], F32)
                            for icc in range(CH):
                                nc.tensor.matmul(
                                    ps, lhsT=wv_sb[:, icc, occ * 128:(occ + 1) * 128],
                                    rhs=zt[:, icc, :], start=(icc == 0), stop=(icc == CH - 1))
                            vt = p1vo.tile([128, PB], BF16, tag="vt")
                            nc.scalar.activation(
                                out=vt, in_=ps, func=mybir.ActivationFunctionType.Identity,
                                bias=bv_sb[:, occ:occ + 1], scale=1.0)
                            nc.gpsimd.dma_start(
                                out=v_out_r[:, occ, pb * PB:(pb + 1) * PB], in_=vt)

                    def qk_block(pb):
                        xt = p1in.tile([128, CH, PB], FP8, tag="xt")
                        nc.sync.dma_start(out=xt, in_=xb_r[:, :, pb * PB:(pb + 1) * PB])
                        yt = p1in.tile([128, CH, PB], FP8, tag="yt")
                        nc.sync.dma_start(out=yt, in_=yb_r[:, :, pb * PB:(pb + 1) * PB])
                        for j in range(4):
                            h = pb * 4 + j
                            ps = qkps.tile([128, 2, C], F32)
                            nc.tensor.matmul(
                                ps[:, 0, :], lhsT=xt[:, :, j * 128:(j + 1) * 128],
                                rhs=wq_sb, start=True, stop=True,
                                perf_mode=mybir.MatmulPerfMode.DoubleRow)
                            nc.tensor.matmul(
                                ps[:, 1, :], lhsT=yt[:, :, j * 128:(j + 1) * 128],
                                rhs=wk_sb, start=True, stop=True,
                                perf_mode=mybir.MatmulPerfMode.DoubleRow)
                            # Q: (ps*COPY_SCALE) + bq  (fused DVE), out fp8
                            nc.vector.scalar_tensor_tensor(
                                out=Q_sb[:, h, :], in0=ps[:, 0, :], scalar=COPY_SCALE,
                                in1=bq_sb, op0=mybir.AluOpType.mult,
                                op1=mybir.AluOpType.add)
                            # K: plain scaled copy on ACT, out fp8 (bk dropped)
                            nc.scalar.activation(
                                out=K_sb[:, h, :], in_=ps[:, 1, :],
                                func=mybir.ActivationFunctionType.Identity,
                                scale=COPY_SCALE)

                    # Front-load V (2 V-blocks per step for 16 steps) so the
                    # v_sb SBUF fill overlaps the QK-only tail instead of
                    # stalling the attention phase.
                    for step in range(16):
                        v_block(2 * step)
                        v_block(2 * step + 1)
                        qk_block(step)
                    for q in range(4):
                        nc.gpsimd.dma_start(
                            out=v_sb[:, q * 64:(q + 1) * 64, 0:W],
                            in_=v_in_r[:, q * 64:(q + 1) * 64, :])
                    for step in range(16, NB):
                        qk_block(step)

                with tc.tile_pool(name="att_e", bufs=4) as epool, \
                     tc.tile_pool(name="att_o", bufs=4) as opool, \
                     tc.tile_pool(name="att_r", bufs=8) as rpool, \
                     tc.tile_pool(name="att_sps", bufs=2, space="PSUM") as sps, \
                     tc.tile_pool(name="att_aps", bufs=6, space="PSUM") as aps:
                    NG = C // 4
                    E_tiles = [None] * NG
                    for g in range(NG + 2):
                        if g < NG:
                            ps_s = sps.tile([128, 4, H], F32)
                            for j in range(4):
                                c = g * 4 + j
                                nc.tensor.matmul(
                                    ps_s[:, j, :], lhsT=K_sb[:, :, c], rhs=Q_sb[:, :, c],
                                    start=True, stop=True)
                            E_sb = epool.tile([128, 4, H], BF16)
                            nc.scalar.activation(
                                out=E_sb, in_=ps_s,
                                func=mybir.ActivationFunctionType.Exp, scale=EXP_SCALE)
                            E_tiles[g] = E_sb
                        if g >= 2:           # 2-group lookahead hides exp latency
                            gp = g - 2
                            E_p = E_tiles[gp]
                            at = opool.tile([128, 4, W + 2], BF16)
                            nc.gpsimd.memset(at[:, :, 0:1], 0.0)
                            nc.gpsimd.memset(at[:, :, W + 1:W + 2], 0.0)
                            for j in range(4):
                                c = gp * 4 + j
                                ps_a = aps.tile([128, W + 1], F32)
                                nc.tensor.matmul(
                                    ps_a, lhsT=E_p[:, j, :], rhs=v_sb[:, c, 0:W + 1],
                                    start=True, stop=True)
                                r = rpool.tile([128, 1], F32)
                                nc.vector.reciprocal(r, ps_a[:, W:W + 1])
                                if j == 3:
                                    nc.scalar.activation(
                                        out=at[:, j, 1:W + 1], in_=ps_a[:, 0:W],
                                        func=mybir.ActivationFunctionType.Identity,
                                        scale=r)
                                else:
                                    nc.vector.tensor_scalar_mul(
                                        out=at[:, j, 1:W + 1], in0=ps_a[:, 0:W],
                                        scalar1=r)
                            c0 = gp * 4
                            nc.sync.dma_start(
                                out=att_dram[c0:c0 + 4, 1:H + 1, :].rearrange(
                                    "c h w -> h c w"),
                                in_=at)
                            E_tiles[gp] = None

            # =============== Phase 3: conv3x3 + BN, occ-outer ================
            with tc.tile_pool(name="conv_store", bufs=1) as cstore:
                o_sb = cstore.tile([128, CH, HW], F32)
                stats_acc = cstore.tile([128, CH, NB, 6], F32)
                st_f = cstore.tile([128, CH], F32)   # scale per occ chunk
                st_t = cstore.tile([128, CH], F32)   # shift per occ chunk

                with tc.tile_pool(name="conv_in", bufs=6) as cin, \
                     tc.tile_pool(name="conv_ps", bufs=4, space="PSUM") as cps, \
                     tc.tile_pool(name="st", bufs=1) as st, \
                     tc.tile_pool(name="st_dram", bufs=1, space="DRAM") as stdram, \
                     tc.tile_pool(name="apply_t", bufs=3) as apool:

                    ar_bufs = []
                    for occ in range(CH):
                        sin = stdram.tile([128, 2], F32, tag=f"sin{occ}")
                        sout = stdram.tile([128, 2], F32, tag=f"sout{occ}")
                        ar_bufs.append((sin, sout))

                    def conv_block(occ, pb):
                        att_t = cin.tile([128, CH, 6, W + 2], BF16, tag="att_t")
                        for icc in range(CH):
                            eng = nc.sync if icc == 0 else nc.scalar
                            eng.dma_start(
                                out=att_t[:, icc, :, :],
                                in_=att_dram[icc * 128:(icc + 1) * 128,
                                             4 * pb:4 * pb + 6, :])
                        ps = cps.tile([128, PB], F32)
                        i_mm = 0
                        for icc in range(CH):
                            for dy in range(3):
                                for dx in range(3):
                                    tsel = (dy * 3 + dx) * CH + icc
                                    nc.tensor.matmul(
                                        ps,
                                        lhsT=wt_sb[:, tsel, occ * 128:(occ + 1) * 128],
                                        rhs=att_t[:, icc, dy:dy + 4, dx:dx + W],
                                        start=(i_mm == 0), stop=(i_mm == 9 * CH - 1))
                                    i_mm += 1
                        nc.vector.bn_stats(out=stats_acc[:, occ, pb, :], in_=ps)
                        nc.scalar.activation(
                            out=o_sb[:, occ, pb * PB:(pb + 1) * PB], in_=ps,
                            func=mybir.ActivationFunctionType.Identity)

                    def stats_chain(occ):
                        # local (sum, sumsq) -> AllReduce (async wrt conv engines)
                        sin, sout = ar_bufs[occ]
                        mv = st.tile([128, 2], F32, tag="mv")
                        nc.vector.bn_aggr(out=mv, in_=stats_acc[:, occ])
                        loc = st.tile([128, 2], F32, tag="loc")
                        # loc0 = mean*HW ; loc1 = (var + mean^2)*HW
                        nc.scalar.mul(out=loc[:, 0:1], in_=mv[:, 0:1], mul=float(HW))
                        msq = st.tile([128, 1], F32, tag="msq")
                        nc.vector.tensor_mul(msq, mv[:, 0:1], mv[:, 0:1])
                        ex2 = st.tile([128, 1], F32, tag="ex2")
                        nc.vector.tensor_add(ex2, mv[:, 1:2], msq)
                        nc.scalar.mul(out=loc[:, 1:2], in_=ex2, mul=float(HW))
                        nc.gpsimd.dma_start(out=sin, in_=loc)
                        nc.gpsimd.collective_compute(
                            "AllReduce", mybir.AluOpType.add,
                            replica_groups=[list(range(N_CORES))],
                            ins=[sin.opt()], outs=[sout.opt()])

                    def finalize_chain(occ):
                        # read AR result, compute scale/shift for this chunk
                        _, sout = ar_bufs[occ]
                        glob = st.tile([128, 2], F32, tag="glob")
                        nc.gpsimd.dma_start(out=glob, in_=sout)
                        mg = st.tile([128, 1], F32, tag="mg")
                        nc.scalar.mul(out=mg, in_=glob[:, 0:1], mul=1.0 / N_TOT)
                        e2g = st.tile([128, 1], F32, tag="e2g")
                        nc.scalar.mul(out=e2g, in_=glob[:, 1:2], mul=1.0 / N_TOT)
                        mg2 = st.tile([128, 1], F32, tag="mg2")
                        nc.vector.tensor_mul(mg2, mg, mg)
                        var = st.tile([128, 1], F32, tag="var")
                        nc.vector.tensor_sub(var, e2g, mg2)
                        sd = st.tile([128, 1], F32, tag="sd")
                        nc.scalar.activation(
                            out=sd, in_=var, func=mybir.ActivationFunctionType.Sqrt,
                            bias=eps_sb, scale=1.0)
                        rsd = st.tile([128, 1], F32, tag="rsd")
                        nc.vector.reciprocal(rsd, sd)
                        nc.vector.tensor_mul(st_f[:, occ:occ + 1], rsd,
                                             g_sb[:, occ:occ + 1])
                        ms = st.tile([128, 1], F32, tag="ms")
                        nc.vector.tensor_mul(ms, mg, st_f[:, occ:occ + 1])
                        nc.vector.tensor_sub(st_t[:, occ:occ + 1],
                                             be_sb[:, occ:occ + 1], ms)

                    def apply_block(occ, pb0, npb, dma_eng):
                        # y = s*x + t ; out = max(0.2*y, y)  == LeakyReLU(y)
                        xin = o_sb[:, occ, pb0 * PB:(pb0 + npb) * PB]
                        yt = apool.tile([128, npb * PB], F32, tag="y")
                        nc.scalar.activation(
                            out=yt, in_=xin,
                            func=mybir.ActivationFunctionType.Identity,
                            scale=st_f[:, occ:occ + 1], bias=st_t[:, occ:occ + 1])
                        ot = apool.tile([128, npb * PB], BF16, tag="ot")
                        nc.vector.scalar_tensor_tensor(
                            out=ot, in0=yt, scalar=LEAKY, in1=yt,
                            op0=mybir.AluOpType.mult, op1=mybir.AluOpType.max)
                        dma_eng.dma_start(
                            out=out[occ * 128:(occ + 1) * 128,
                                    pb0 * PB:(pb0 + npb) * PB],
                            in_=ot)

                    # occ 0 conv
                    for pb in range(NB):
                        conv_block(0, pb)
                    stats_chain(0)
                    # occ 1 conv, with occ0 finalize+apply woven in late enough
                    # that the AllReduce is certain to have completed (its
                    # waits would otherwise stall conv's PSUM-freeing copies
                    # queued behind them on ACT/DVE).
                    a0 = 0
                    for pb in range(NB):
                        conv_block(1, pb)
                        if pb == 26:
                            finalize_chain(0)
                        if pb > 26:
                            for _ in range(3):
                                if a0 < 16:
                                    apply_block(0, 2 * a0, 2, nc.gpsimd)
                                    a0 += 1
                    stats_chain(1)
                    while a0 < 16:
                        apply_block(0, 2 * a0, 2, nc.gpsimd)
                        a0 += 1
                    finalize_chain(1)
                    for i in range(8):
                        apply_block(1, 4 * i, 4, nc.sync if i % 2 else nc.gpsimd)
    return nc


_NC_CACHE = None


def _get_nc():
    global _NC_CACHE
    if _NC_CACHE is None:
        _NC_CACHE = _build()
    return _NC_CACHE


def kernel(x, y, z, wq, bq, wk, bk, wv, bv, w_out, b_out, gamma, beta, **_unused):
    x = np.asarray(x, dtype=np.float32)
    y = np.asarray(y, dtype=np.float32)
    z = np.asarray(z, dtype=np.float32)
    scale = 1.0 / math.sqrt(W)

    wq8 = np.ascontiguousarray(
        (np.asarray(wq, np.float32).T * (scale * SW)).astype(nfp8))
    wk8 = np.ascontiguousarray((np.asarray(wk, np.float32).T * SW).astype(nfp8))
    wvT = np.ascontiguousarray(np.asarray(wv, np.float32).T.astype(nbf16))
    bqb = np.tile((np.asarray(bq, np.float32) * (scale * SQ))[None, :],
                  (128, 1)).astype(np.float32)
    bvh = np.asarray(bv, np.float32).reshape(C, 1)
    # w_out [oc, ic, 3, 3] -> wtap[(dy*3+dx)*CH + icc][ic(128), oc]
    wo = np.asarray(w_out, np.float32)
    wtap = np.empty((9 * CH, 128, C), dtype=nbf16)
    for dy in range(3):
        for dx in range(3):
            wt = wo[:, :, dy, dx].T  # [ic, oc]
            for icc in range(CH):
                wtap[(dy * 3 + dx) * CH + icc] = wt[icc * 128:(icc + 1) * 128].astype(nbf16)
    gm = np.asarray(gamma, np.float32).reshape(C, 1)
    bt = np.asarray(beta, np.float32).reshape(C, 1)

    shared = dict(wq8=wq8, wk8=wk8, wvT=wvT, bqb=bqb, bv=bvh,
                  wtap=wtap, gamma=gm, beta=bt)
    in_maps = []
    for i in range(N_CORES):
        in_maps.append(dict(
            xb=(x[i].reshape(C, HW) * SX).astype(nfp8),
            yb=(y[i].reshape(C, HW) * SX).astype(nfp8),
            zb=z[i].reshape(C, HW).astype(nbf16),
            **shared))

    nc = _get_nc()
    global _last_in_maps
    _last_in_maps = in_maps
    res = run_bass_kernel_spmd(nc, in_maps, list(range(N_CORES)))
    out = np.stack([np.asarray(res.results[i]["out"]).astype(np.float32)
                    .reshape(C, H, W) for i in range(N_CORES)])
    return out


if __name__ == "__main__":
    pass
